# revision 1
# baseline (speedup 1.0000x reference)
"""Trainium2 Bass kernel for EquivariantSubSampling.

The reference module reduces to a per-batch gather (verified numerically):
with (oh, ow, r) = p[b] (each in {0,1}), ic = 2*oc + r:
    r=0: out[b, oc, a, c] = x[b, ic, oh + 2a, ow + 2c]
    r=1: out[b, oc, a, c] = x[b, ic, oh + 2*((32-c) % 32), ow + 2a]

Strategy: pure data parallel over the batch dim (16 batches / 8 cores = 2
per core).  Raw bacc program (no Tile framework — avoids its multi-us
preamble/teardown barriers).  Per batch, on device:
  - the p-derived scalars arrive as a tiny host-marshalled int32 input q
    ([oh0, r0, oh1, r1, ow0, ow1]); engines read them into registers
    straight from HBM (no staging DMA), two values at a time so the
    first input DMA issues as early as possible
  - the needed input rows x[b, r::2, oh::2, :] are loaded with
    register-offset (dynamic) DMAs, the row halves split across the two
    HWDGE rings (sync + scalar engines), one semaphore per half so
    compute can start when the first half lands
  - both gather variants are computed unconditionally into one tile
    (V[:, 0] = r0-variant, V[:, 1] = r1-variant), staged per input half
    and split across the vector and scalar engines; the output DMA then
    reads V[:, ds(r, 1)] (dynamic SBUF offset) — a branchless select
  - gpsimd clears the semaphores at the end so the NEFF is re-executable

Gather geometry per batch (A = SBUF copy of the 32 needed rows):
  V0[a, c] = A[a, ow + 2c]                      (r=0 variant)
  V1[a, c] = A[(32 - c) % 32, ow + 2a]          (r=1 variant)
  stage A (rows 0:16):  copy0 a in [0,16);  copy1 c in [17,32) + c == 0
  stage B (rows 16:32): copy0 a in [16,32); copy1 c in [1,17)
"""

import numpy as np

B, C, H, W = 16, 256, 64, 64
NCORES = 8
BPC = B // NCORES           # batches per core
OC, OHW = 128, 32           # output channels, output spatial

_COMPILED = {}


def build_nc(enable_asserts=False):
    RS = 16
    from contextlib import ExitStack

    import concourse.bacc as bacc
    import concourse.bass as bass
    import concourse.mybir as mybir

    ds = bass.ds
    f32 = mybir.dt.float32
    i32 = mybir.dt.int32
    ET = mybir.EngineType

    nc = bacc.Bacc(
        "TRN2",
        target_bir_lowering=False,
        debug=False,
        enable_asserts=enable_asserts,
        num_devices=NCORES,
    )
    x_d = nc.dram_tensor("x", [BPC, C, H, W], f32, kind="ExternalInput").ap()
    # q = host-marshalled p: [oh0, r0, oh1, r1, ow0, ow1]
    q_d = nc.dram_tensor("q", [1, 3 * BPC], i32, kind="ExternalInput").ap()
    o_d = nc.dram_tensor("out", [BPC, OC, OHW, OHW], f32, kind="ExternalOutput").ap()

    with ExitStack() as ctx:
        e = ctx.enter_context
        ow_sb = e(nc.sbuf_tensor("ow_sb", [1, BPC], i32)).ap()
        a_sb = [
            e(nc.sbuf_tensor(f"a_sb{b}", [128, 32 * 64], f32)) for b in range(BPC)
        ]
        v_sb = [
            e(nc.sbuf_tensor(f"v_sb{b}", [128, 2, OHW * OHW], f32))
            for b in range(BPC)
        ]
        s_p = e(nc.semaphore(name="s_p"))
        s_lo = [e(nc.semaphore(name=f"s_lo{b}")) for b in range(BPC)]
        s_hi = [e(nc.semaphore(name=f"s_hi{b}")) for b in range(BPC)]
        s_c = [e(nc.semaphore(name=f"s_c{b}")) for b in range(BPC)]
        s_out = e(nc.semaphore(name="s_out"))
        s_out2 = e(nc.semaphore(name="s_out2"))
        all_sems = [s_p, *s_lo, *s_hi, *s_c, s_out, s_out2]

        a_v = [t.ap().rearrange("p (r c) -> p r c", r=32) for t in a_sb]
        v_v = [t.ap() for t in v_sb]
        v0 = [v[:, 0, :].rearrange("p (a c) -> p a c", a=OHW) for v in v_v]
        v1 = [v[:, 1, :].rearrange("p (a c) -> p a c", a=OHW) for v in v_v]

        def load_vals(engine_type, src, lo, hi):
            _, vals = nc.values_load_multi_w_load_instructions(
                src[0:1, lo:hi],
                engines=[engine_type],
                min_val=0,
                max_val=1,
                skip_runtime_bounds_check=True,
            )
            return vals

        def wait_all_sems(eng):
            # the race validator requires every engine to observe every
            # semaphore's final value before the end-of-kernel clear
            eng.wait_ge(s_p, 16)
            for b in range(BPC):
                eng.wait_ge(s_lo[b], 16)
                eng.wait_ge(s_hi[b], 16)
                eng.wait_ge(s_c[b], 2)
            eng.wait_ge(s_out, 16 * (BPC - 1))
            eng.wait_ge(s_out2, 32)

        block = e(nc.Block(no_gpsimd_drain=True))

        @block.sync
        def _(sync):
            # all DRAM register loads happen before any DMA traffic starts —
            # engine loads from HBM during active DMA streaming take 2-3x
            # longer and stall the issue chain
            ohr4 = load_vals(ET.SP, q_d, 0, 2 * BPC)
            ohr = [(ohr4[2 * b], ohr4[2 * b + 1]) for b in range(BPC)]
            for b in range(BPC):
                oh, r = ohr[b]
                sync.dma_start(
                    a_v[b][:, 0:RS, :],
                    x_d[b][ds(r, 128, 2), ds(oh, RS, 2), :],
                ).then_inc(s_lo[b], 16)
            # last batch's output on the (by now idle) HWDGE ring — lower
            # first-byte latency than SWDGE
            rlast = ohr[BPC - 1][1]
            sync.wait_ge(s_c[BPC - 1], 2)
            sync.dma_start(
                o_d[BPC - 1][:, 0:16, :].rearrange("c h w -> c (h w)").unsqueeze(1),
                v_v[BPC - 1][:, ds(rlast, 1), 0:512],
            ).then_inc(s_out2, 16)
            wait_all_sems(sync)
            sync.drain()

        @block.scalar
        def _(scalar):
            ohr4 = load_vals(ET.Activation, q_d, 0, 2 * BPC)
            # stage ow values into SBUF for ACT/DVE (rides first on the ring)
            scalar.dma_start(ow_sb[:], q_d[0:1, 2 * BPC : 3 * BPC]).then_inc(s_p, 16)
            for b in range(BPC):
                oh, r = ohr4[2 * b], ohr4[2 * b + 1]
                scalar.dma_start(
                    a_v[b][:, RS:32, :],
                    x_d[b][ds(r, 128, 2), ds(oh + 2 * RS, 32 - RS, 2), :],
                ).then_inc(s_hi[b], 16)
            scalar.wait_ge(s_p, 16)
            ows = load_vals(ET.Activation, ow_sb, 0, BPC)
            for b in range(BPC):
                ow = ows[b]
                # hi stage first — the hi chunks land before the lo chunks
                # stage B (ACT share): c 1:9 (rows 31..24)
                scalar.wait_ge(s_hi[b], 16)
                scalar.copy(
                    v1[b][:, :, 1:9],
                    a_v[b][:, 31:23:-1, ds(ow, 32, 2)].transpose([0, 2, 1]),
                )
                # stage A (ACT share): c=0 strip (row 0) + c 17:25 (rows 15..8)
                scalar.wait_ge(s_lo[b], 16)
                scalar.copy(
                    v1[b][:, :, 0:1],
                    a_v[b][:, 0:1, ds(ow, 32, 2)].transpose([0, 2, 1]),
                )
                scalar.copy(
                    v1[b][:, :, 17:22],
                    a_v[b][:, 15:10:-1, ds(ow, 32, 2)].transpose([0, 2, 1]),
                ).then_inc(s_c[b], 1)
            rlast = ohr4[2 * BPC - 1]
            scalar.wait_ge(s_c[BPC - 1], 2)
            scalar.dma_start(
                o_d[BPC - 1][:, 16:32, :].rearrange("c h w -> c (h w)").unsqueeze(1),
                v_v[BPC - 1][:, ds(rlast, 1), 512:1024],
            ).then_inc(s_out2, 16)
            wait_all_sems(scalar)
            scalar.drain()

        @block.vector
        def _(vector):
            vector.wait_ge(s_p, 16)
            ows = load_vals(ET.DVE, ow_sb, 0, BPC)
            for b in range(BPC):
                ow = ows[b]
                # stage B first: copy0 a 16:32 + copy1 c 9:17 (rows 23..16)
                vector.wait_ge(s_hi[b], 16)
                vector.tensor_copy(
                    v0[b][:, 16:32, :], a_v[b][:, 16:32, ds(ow, 32, 2)]
                )
                vector.tensor_copy(
                    v1[b][:, :, 9:17],
                    a_v[b][:, 23:15:-1, ds(ow, 32, 2)].transpose([0, 2, 1]),
                )
                # stage A: copy0 a 0:16 + copy1 c 25:32 (rows 7..1)
                vector.wait_ge(s_lo[b], 16)
                vector.tensor_copy(
                    v0[b][:, 0:16, :], a_v[b][:, 0:16, ds(ow, 32, 2)]
                )
                vector.tensor_copy(
                    v1[b][:, :, 22:32],
                    a_v[b][:, 10:0:-1, ds(ow, 32, 2)].transpose([0, 2, 1]),
                ).then_inc(s_c[b], 1)
            wait_all_sems(vector)
            vector.drain()

        @block.tensor
        def _(tensor):
            wait_all_sems(tensor)

        @block.gpsimd
        def _(gpsimd):
            # output DMAs on the SWDGE ring so the two HWDGE rings carry
            # only input traffic (dynamic select between the two variants)
            ohr4 = load_vals(ET.Pool, q_d, 0, 2 * BPC)
            for b in range(BPC - 1):
                r = ohr4[2 * b + 1]
                gpsimd.wait_ge(s_c[b], 2)
                gpsimd.dma_start(
                    o_d[b].rearrange("c h w -> c (h w)").unsqueeze(1),
                    v_v[b][:, ds(r, 1), :],
                ).then_inc(s_out, 16)

            wait_all_sems(gpsimd)
            nums = sorted(s.num for s in all_sems)
            rng = range(nums[0], nums[-1] + 1)
            gpsimd.dma_reset(rng)
            gpsimd.sem_clear(rng)

    nc.compile()
    return nc


def make_in_maps(x, p):
    x = np.ascontiguousarray(x, dtype=np.float32)
    p = np.ascontiguousarray(p, dtype=np.int32)
    assert x.shape == (B, C, H, W) and p.shape == (B, 3)
    in_maps = []
    for i in range(NCORES):
        pc = p[i * BPC : (i + 1) * BPC]
        q = np.empty((1, 3 * BPC), np.int32)
        for b in range(BPC):
            q[0, 2 * b] = pc[b, 0]      # oh
            q[0, 2 * b + 1] = pc[b, 2]  # r
            q[0, 2 * BPC + b] = pc[b, 1]  # ow
        in_maps.append({"x": x[i * BPC : (i + 1) * BPC], "q": q})
    return in_maps


def _get_nc():
    if "nc" not in _COMPILED:
        _COMPILED["nc"] = build_nc()
    return _COMPILED["nc"]


def kernel(x: np.ndarray, p: np.ndarray) -> np.ndarray:
    from concourse.bass_utils import run_bass_kernel_spmd

    nc = _get_nc()
    res = run_bass_kernel_spmd(nc, make_in_maps(x, p), core_ids=list(range(NCORES)))
    return np.concatenate(
        [res.results[i]["out"] for i in range(NCORES)], axis=0
    )

